# revision 4
# baseline (speedup 1.0000x reference)
"""Bass/Tile TRN2 kernel for nn_Attention_12704513261709.

Algebraic reduction: per head h (dh=2048 > d=256), fold the projections into
two 256x256 matrices on the host:
    M'_h = diag(1+gamma) . (scale . Wq_h^T Wk_h) . diag(1+gamma)
    P'_h = (Wo[:, h] . Wv_h) . diag(1+gamma)
Then with xh = plain layernorm(x) (no gamma):
    S_h   = xh M'_h xh^T                (logits, per batch)
    out   = sum_h softmax(S_h) xh P'_h^T
This cuts per-core PE work ~9x vs materializing q/k/v in dh=2048.

8-way head-parallel: core h computes head h for both batches; host sums the
8 partial outputs. Per-core device pipeline: LN -> xnT via XBAR DMA
transpose -> Y^T = M'^T xn^T, Z = xn P'^T (+ ones column for the softmax
row-sum) -> per 512-query chunk: S^T tile pairs in 2-bank PSUM -> one exp
per pair -> A^T @ [Z|1] accumulated over key tiles (rowsum rides along as
column 256) -> scale by 1/rowsum -> DMA out.

Shapes: x (2,2048,256) f32, gamma (256,), Wq/Wk/Wv (16384,256), Wo (256,16384).
"""

import numpy as np
import ml_dtypes

B = 2
N_SEQ = 2048
N_TOK = B * N_SEQ  # 4096
D = 256
HEADS = 8
DH = 2048  # per-head dim of the original module (16384/8)
SCALE = 64 ** (-0.5)
EPS = 1e-5

TT = N_SEQ // 128  # 16 key tiles per batch
NCH = N_SEQ // 512  # 4 query chunks of 512 per batch
NK = TT // 2  # 8 S-tile pairs per chunk

_CACHE = {}


def _build():
    from concourse import bacc
    import concourse.tile as tile
    import concourse.mybir as mybir

    f32 = mybir.dt.float32
    bf16 = mybir.dt.bfloat16
    AF = mybir.ActivationFunctionType
    ALU = mybir.AluOpType

    nc = bacc.Bacc("TRN2", target_bir_lowering=False, debug=False, num_devices=8)

    x_d = nc.dram_tensor("x", [N_TOK, D], f32, kind="ExternalInput").ap()
    m_d = nc.dram_tensor("m", [D, D], bf16, kind="ExternalInput").ap()
    pT_d = nc.dram_tensor("pT", [D, D], bf16, kind="ExternalInput").ap()
    o_d = nc.dram_tensor("o_part", [N_TOK, D], f32, kind="ExternalOutput").ap()

    with tile.TileContext(nc) as tc:
        with (
            tc.tile_pool(name="singles", bufs=1) as singles,
            tc.tile_pool(name="ln", bufs=8) as ln_pool,
            tc.tile_pool(name="stage", bufs=4) as stage_pool,
            tc.tile_pool(name="ptp", bufs=1) as pt_pool,
            tc.tile_pool(name="psS", bufs=2, space="PSUM") as psS,
            tc.tile_pool(name="psO", bufs=2, space="PSUM") as psO,
        ):
            eps_t = singles.tile([128, 1], f32)
            nc.vector.memset(eps_t, EPS)

            # dummy matmuls keep the PE clock-gate warm during the prologue
            dummy_w = singles.tile([128, 128], bf16)
            nc.vector.memset(dummy_w, 0.0)
            dummy_r = singles.tile([128, 256], bf16)
            nc.vector.memset(dummy_r, 0.0)

            def dummy_mm():
                ps = psS.tile([128, 1024], f32, tag="s", name="warm")
                nc.tensor.matmul(ps[:, :256], dummy_w[:], dummy_r[:], start=True, stop=True)

            for _ in range(16):
                dummy_mm()

            # small folded weights
            m_sb = [singles.tile([128, D], bf16, name=f"msb{i}") for i in range(2)]
            pT_sb = [singles.tile([128, D], bf16, name=f"ptsb{i}") for i in range(2)]

            xnT = [
                [singles.tile([128, N_SEQ], bf16, name=f"xnT{bb}{d_}") for d_ in range(2)]
                for bb in range(B)
            ]
            yT = [
                [singles.tile([128, N_SEQ], bf16, name=f"yT{bb}{d_}") for d_ in range(2)]
                for bb in range(B)
            ]
            # Z' = [xh P'^T | 1]: per key tile, 257 cols (col 256 = ones)
            zp = [
                [singles.tile([128, 257], bf16, name=f"zp{bb}{t}") for t in range(TT)]
                for bb in range(B)
            ]
            for bb in range(B):
                for t in range(TT):
                    nc.gpsimd.memset(zp[bb][t][:, 256:257], 1.0)

            state = {}

            def ln_chain(bb, i):
                """LayerNorm token tile (bb, i): DMA + DVE/ACT chain -> bf16 xn."""
                gi = bb * TT + i
                x_t = ln_pool.tile([128, D], f32, tag="x", name="x")
                nc.sync.dma_start(x_t[:], x_d[gi * 128 : (gi + 1) * 128, :])
                stats = ln_pool.tile([128, nc.vector.BN_STATS_DIM], f32, tag="st", name="st")
                nc.vector.bn_stats(stats[:], x_t[:])
                mv = ln_pool.tile([128, nc.vector.BN_AGGR_DIM], f32, tag="mv", name="mv")
                nc.vector.bn_aggr(mv[:], stats[:])
                std = ln_pool.tile([128, 1], f32, tag="sd", name="sd")
                nc.scalar.activation(std[:], mv[:, 1:2], func=AF.Sqrt, bias=eps_t[:], scale=1.0)
                rstd = ln_pool.tile([128, 1], f32, tag="rs", name="rs")
                nc.vector.reciprocal(rstd[:], std[:])
                xn_t = ln_pool.tile([128, D], bf16, tag="xn", name="xn")
                nc.vector.tensor_scalar(
                    xn_t[:],
                    x_t[:],
                    scalar1=mv[:, 0:1],
                    scalar2=rstd[:],
                    op0=ALU.subtract,
                    op1=ALU.mult,
                )
                state[gi % 8] = xn_t

            def tp_dma(bb, i):
                """Transpose xn tile into xnT via the XBAR DMA path (no PE)."""
                gi = bb * TT + i
                xn_t = state[gi % 8]
                for d_ in range(2):
                    nc.scalar.dma_start_transpose(
                        xnT[bb][d_][:, i * 128 : (i + 1) * 128],
                        xn_t[:, d_ * 128 : (d_ + 1) * 128],
                    )

            def ln_tile(bb, i):
                ln_chain(bb, i)
                tp_dma(bb, i)

            def z_build(bb, t):
                """Z rows for key tile t: [128 tok, 256] = xn_tile @ P'^T."""
                ps = psS.tile([128, 1024], f32, tag="s", name="zps")
                for d_ in range(2):
                    nc.tensor.matmul(
                        ps[:, :256],
                        xnT[bb][d_][:, t * 128 : (t + 1) * 128],
                        pT_sb[d_][:],
                        start=(d_ == 0),
                        stop=(d_ == 1),
                    )
                nc.vector.tensor_copy(zp[bb][t][:, :256], ps[:, :256])

            def yt_build(bb, c, dm):
                """Y^T[dm-half, 512-query chunk c] = M'^T xn^T."""
                ps = psS.tile([128, 1024], f32, tag="s", name="yps")
                for d_ in range(2):
                    nc.tensor.matmul(
                        ps[:, :512],
                        m_sb[d_][:, dm * 128 : (dm + 1) * 128],
                        xnT[bb][d_][:, c * 512 : (c + 1) * 512],
                        start=(d_ == 0),
                        stop=(d_ == 1),
                    )
                nc.scalar.copy(yT[bb][dm][:, c * 512 : (c + 1) * 512], ps[:, :512])

            def av_pair(bb, k, pt_k, av_ps):
                for half in range(2):
                    t = 2 * k + half
                    for qt in range(4):
                        ti, h2 = divmod(qt, 2)
                        nc.tensor.matmul(
                            av_ps[ti][:, h2 * 512 : h2 * 512 + 257],
                            pt_k[:, half * 512 + qt * 128 : half * 512 + (qt + 1) * 128],
                            zp[bb][t][:, :257],
                            start=(t == 0),
                            stop=(t == TT - 1),
                        )

            def chunk(bb, ch, pre=None):
                """One 512-query chunk: S^T pairs -> exp -> AV (interleaved),
                then 1/rowsum scale + output DMA. pre[k] = callables to weave
                before S-pair k."""
                cq = ch * 512
                gbase = bb * N_SEQ + cq
                av_ps = [
                    psO.tile([128, 1024], f32, tag="o", name=f"av{ti}") for ti in range(2)
                ]
                pts = []
                for k in range(NK):
                    sps = psS.tile([128, 1024], f32, tag="s", name="sps")
                    for half in range(2):
                        t = 2 * k + half
                        for d_ in range(2):
                            nc.tensor.matmul(
                                sps[:, half * 512 : (half + 1) * 512],
                                xnT[bb][d_][:, t * 128 : (t + 1) * 128],
                                yT[bb][d_][:, cq : cq + 512],
                                start=(d_ == 0),
                                stop=(d_ == 1),
                            )
                    pt_k = pt_pool.tile([128, 1024], bf16, tag=f"pt{k}", name=f"pt{k}")
                    nc.scalar.activation(pt_k[:], sps[:], func=AF.Exp)
                    pts.append(pt_k)
                    if k > 0:
                        av_pair(bb, k - 1, pts[k - 1], av_ps)
                    if pre is not None and k < len(pre):
                        for fn in pre[k]:
                            fn()
                av_pair(bb, NK - 1, pts[NK - 1], av_ps)
                for qt in range(4):
                    ti, h2 = divmod(qt, 2)
                    col = h2 * 512
                    rcp = stage_pool.tile([128, 1], f32, tag="rcp", name="rcp")
                    nc.vector.reciprocal(rcp[:], av_ps[ti][:, col + 256 : col + 257])
                    ob = stage_pool.tile([128, 256], f32, tag="ob", name="ob")
                    nc.vector.tensor_scalar(
                        ob[:], av_ps[ti][:, col : col + 256], scalar1=rcp[:],
                        scalar2=None, op0=ALU.mult,
                    )
                    nc.gpsimd.dma_start(
                        o_d[gbase + qt * 128 : gbase + (qt + 1) * 128, :], ob[:]
                    )

            # ---- prologue: batch-0 LN + transposes (PE-free), first Y^T/Z ----
            for i in range(4):
                ln_chain(0, i)
            for i in range(2):
                nc.gpsimd.dma_start(m_sb[i][:], m_d[i * 128 : (i + 1) * 128, :])
                nc.gpsimd.dma_start(pT_sb[i][:], pT_d[i * 128 : (i + 1) * 128, :])
            for i in range(4):
                tp_dma(0, i)
            for j in range(4, TT):
                ln_chain(0, j)
                tp_dma(0, j)
            yt_build(0, 0, 0)
            yt_build(0, 0, 1)
            z_build(0, 0)
            z_build(0, 1)

            # ---- main loop; remaining prep woven into the chunk PE stream ----
            def mk(f, *a):
                return lambda: f(*a)

            pre00 = [[] for _ in range(NK)]
            for k in range(7):  # z tiles 2..15, two per slot
                pre00[k] += [mk(z_build, 0, 2 + 2 * k), mk(z_build, 0, 3 + 2 * k)]
            pre00[3] += [mk(yt_build, 0, 1, 0), mk(yt_build, 0, 1, 1)]
            pre00[5] += [mk(yt_build, 0, 2, 0), mk(yt_build, 0, 2, 1)]
            pre00[7] += [mk(yt_build, 0, 3, 0), mk(yt_build, 0, 3, 1)]
            pre01 = [[mk(ln_tile, 1, 2 * k), mk(ln_tile, 1, 2 * k + 1)] for k in range(NK)]
            pre02 = [[mk(z_build, 1, 2 * k), mk(z_build, 1, 2 * k + 1)] for k in range(NK)]
            pre03 = [[mk(yt_build, 1, k // 2, k % 2)] for k in range(NK)]
            pre = {(0, 0): pre00, (0, 1): pre01, (0, 2): pre02, (0, 3): pre03}
            for bb in range(B):
                for ch in range(NCH):
                    chunk(bb, ch, pre.get((bb, ch)))

    nc.compile()
    return nc


def get_nc():
    if "nc" not in _CACHE:
        _CACHE["nc"] = _build()
    return _CACHE["nc"]


def make_in_maps(x, gamma, Wq, Wk, Wv, Wo):
    bf = ml_dtypes.bfloat16
    g = 1.0 + gamma.astype(np.float64)
    x_flat = np.ascontiguousarray(x.reshape(N_TOK, D).astype(np.float32))
    Wq64, Wk64, Wv64, Wo64 = (a.astype(np.float64) for a in (Wq, Wk, Wv, Wo))
    in_maps = []
    for h in range(HEADS):
        sl = slice(h * DH, (h + 1) * DH)
        M = SCALE * (Wq64[sl].T @ Wk64[sl]) * g[:, None] * g[None, :]
        PT = ((Wo64[:, sl] @ Wv64[sl]) * g[None, :]).T
        in_maps.append(
            {
                "x": x_flat,
                "m": np.ascontiguousarray(M.astype(bf)),
                "pT": np.ascontiguousarray(PT.astype(bf)),
            }
        )
    return in_maps


def gather(results):
    acc = np.zeros((N_TOK, D), np.float32)
    for h in range(HEADS):
        acc += results[h]["o_part"]
    return acc.reshape(B, N_SEQ, D)


def kernel(x, gamma, Wq, Wk, Wv, Wo):
    from concourse import bass_utils

    x, gamma, Wq, Wk, Wv, Wo = (
        np.asarray(a) for a in (x, gamma, Wq, Wk, Wv, Wo)
    )
    nc = get_nc()
    in_maps = make_in_maps(x, gamma, Wq, Wk, Wv, Wo)
    res = bass_utils.run_bass_kernel_spmd(
        nc, in_maps, core_ids=list(range(HEADS))
    )
    return gather(res.results).astype(np.float32)


# revision 6
# speedup vs baseline: 1.3704x; 1.3704x over previous
"""Bass/Tile TRN2 kernel for nn_Attention_12704513261709.

Algebraic reduction: per head h (dh=2048 > d=256), fold the projections into
two 256x256 matrices on the host:
    M'_h = diag(1+gamma) . (scale . Wq_h^T Wk_h) . diag(1+gamma)
    P'_h = (Wo[:, h] . Wv_h) . diag(1+gamma)
Then with xh = plain layernorm(x) (no gamma):
    S_h   = xh M'_h xh^T                (logits, per batch)
    out   = sum_h softmax(S_h) xh P'_h^T
This cuts per-core PE work ~9x vs materializing q/k/v in dh=2048.

8-way head-parallel: core h computes head h for both batches; host sums the
8 partial outputs. Per-core device pipeline: LN -> xnT via PE transpose ->
Y^T = M'^T xn^T, Z = xn P'^T (+ ones column for the softmax row-sum) ->
per 512-query chunk: S^T tiles -> exp -> A^T @ [Z|1] accumulated over key
tiles (rowsum rides along as column 256) -> scale by 1/rowsum -> DMA out.
Engine split: PE matmuls, ACT exp only, DVE layernorm + z/y copies +
normalize, GpSimd transpose copies + output DMA, SP x-load DMA.

Shapes: x (2,2048,256) f32, gamma (256,), Wq/Wk/Wv (16384,256), Wo (256,16384).
"""

import numpy as np
import ml_dtypes

B = 2
N_SEQ = 2048
N_TOK = B * N_SEQ  # 4096
D = 256
HEADS = 8
DH = 2048  # per-head dim of the original module (16384/8)
SCALE = 64 ** (-0.5)
EPS = 1e-5

TT = N_SEQ // 128  # 16 key tiles per batch
NCH = N_SEQ // 512  # 4 query chunks of 512 per batch

_CACHE = {}


def _build():
    from concourse import bacc
    import concourse.tile as tile
    import concourse.mybir as mybir
    from concourse.masks import make_identity

    f32 = mybir.dt.float32
    bf16 = mybir.dt.bfloat16
    AF = mybir.ActivationFunctionType
    ALU = mybir.AluOpType

    nc = bacc.Bacc("TRN2", target_bir_lowering=False, debug=False, num_devices=8)

    x_d = nc.dram_tensor("x", [N_TOK, D], f32, kind="ExternalInput").ap()
    m_d = nc.dram_tensor("m", [D, D], bf16, kind="ExternalInput").ap()
    pT_d = nc.dram_tensor("pT", [D, D], bf16, kind="ExternalInput").ap()
    o_d = nc.dram_tensor("o_part", [N_TOK, D], f32, kind="ExternalOutput").ap()

    with tile.TileContext(nc) as tc:
        with (
            tc.tile_pool(name="singles", bufs=1) as singles,
            tc.tile_pool(name="ln", bufs=8) as ln_pool,
            tc.tile_pool(name="stage", bufs=4) as stage_pool,
            tc.tile_pool(name="ptp", bufs=1) as pt_pool,
            tc.tile_pool(name="psS", bufs=2, space="PSUM") as psS,
            tc.tile_pool(name="psO", bufs=4, space="PSUM") as psO,
            tc.tile_pool(name="psT", bufs=2, space="PSUM") as psT,
        ):
            identity = singles.tile([128, 128], bf16)
            make_identity(nc, identity)
            eps_t = singles.tile([128, 1], f32)
            nc.vector.memset(eps_t, EPS)

            # dummy matmuls keep the PE clock-gate warm during the prologue
            dummy_w = singles.tile([128, 128], bf16)
            nc.vector.memset(dummy_w, 0.0)
            dummy_r = singles.tile([128, 256], bf16)
            nc.vector.memset(dummy_r, 0.0)

            def dummy_mm():
                ps = psS.tile([128, 512], f32, tag="s", name="warm")
                nc.tensor.matmul(ps[:, :256], dummy_w[:], dummy_r[:], start=True, stop=True)

            for _ in range(16):
                dummy_mm()

            # small folded weights
            m_sb = [singles.tile([128, D], bf16, name=f"msb{i}") for i in range(2)]
            pT_sb = [singles.tile([128, D], bf16, name=f"ptsb{i}") for i in range(2)]

            xnT = [
                [singles.tile([128, N_SEQ], bf16, name=f"xnT{bb}{d_}") for d_ in range(2)]
                for bb in range(B)
            ]
            yT = [
                [singles.tile([128, N_SEQ], bf16, name=f"yT{bb}{d_}") for d_ in range(2)]
                for bb in range(B)
            ]
            # Z' = [xh P'^T | 1]: per key tile, 257 cols (col 256 = ones)
            zp = [
                [singles.tile([128, 257], bf16, name=f"zp{bb}{t}") for t in range(TT)]
                for bb in range(B)
            ]
            for bb in range(B):
                for t in range(TT):
                    nc.gpsimd.memset(zp[bb][t][:, 256:257], 1.0)

            state = {}

            def ln_chain(bb, i):
                """LayerNorm token tile (bb, i): DMA + DVE/ACT chain -> bf16 xn."""
                gi = bb * TT + i
                x_t = ln_pool.tile([128, D], f32, tag="x", name="x")
                nc.sync.dma_start(x_t[:], x_d[gi * 128 : (gi + 1) * 128, :])
                stats = ln_pool.tile([128, nc.vector.BN_STATS_DIM], f32, tag="st", name="st")
                nc.vector.bn_stats(stats[:], x_t[:])
                mv = ln_pool.tile([128, nc.vector.BN_AGGR_DIM], f32, tag="mv", name="mv")
                nc.vector.bn_aggr(mv[:], stats[:])
                std = ln_pool.tile([128, 1], f32, tag="sd", name="sd")
                nc.scalar.activation(std[:], mv[:, 1:2], func=AF.Sqrt, bias=eps_t[:], scale=1.0)
                rstd = ln_pool.tile([128, 1], f32, tag="rs", name="rs")
                nc.vector.reciprocal(rstd[:], std[:])
                xn_t = ln_pool.tile([128, D], bf16, tag="xn", name="xn")
                nc.vector.tensor_scalar(
                    xn_t[:],
                    x_t[:],
                    scalar1=mv[:, 0:1],
                    scalar2=rstd[:],
                    op0=ALU.subtract,
                    op1=ALU.mult,
                )
                state[gi % 8] = xn_t

            def ln_transpose(bb, i):
                gi = bb * TT + i
                xn_t = state[gi % 8]
                tp = psT.tile([128, 256], bf16, tag="tp", name="tp")
                for d_ in range(2):
                    nc.tensor.transpose(
                        tp[:, d_ * 128 : (d_ + 1) * 128],
                        xn_t[:, d_ * 128 : (d_ + 1) * 128],
                        identity[:],
                    )
                for d_ in range(2):
                    nc.vector.tensor_copy(
                        xnT[bb][d_][:, i * 128 : (i + 1) * 128],
                        tp[:, d_ * 128 : (d_ + 1) * 128],
                    )

            def ln_tile(bb, i):
                ln_chain(bb, i)
                ln_transpose(bb, i)

            def z_build(bb, t):
                """Z rows for key tile t: [128 tok, 256] = xn_tile @ P'^T."""
                ps = psS.tile([128, 512], f32, tag="s", name="zps")
                for d_ in range(2):
                    nc.tensor.matmul(
                        ps[:, :256],
                        xnT[bb][d_][:, t * 128 : (t + 1) * 128],
                        pT_sb[d_][:],
                        start=(d_ == 0),
                        stop=(d_ == 1),
                    )
                nc.vector.tensor_copy(zp[bb][t][:, :256], ps[:, :256])

            def yt_build(bb, c, dm):
                """Y^T[dm-half, 512-query chunk c] = M'^T xn^T."""
                ps = psS.tile([128, 512], f32, tag="s", name="yps")
                for d_ in range(2):
                    nc.tensor.matmul(
                        ps[:],
                        m_sb[d_][:, dm * 128 : (dm + 1) * 128],
                        xnT[bb][d_][:, c * 512 : (c + 1) * 512],
                        start=(d_ == 0),
                        stop=(d_ == 1),
                    )
                nc.vector.tensor_copy(yT[bb][dm][:, c * 512 : (c + 1) * 512], ps[:])

            def av_step(bb, t, pt_t, av_ps):
                for qt in range(4):
                    nc.tensor.matmul(
                        av_ps[qt][:, :257],
                        pt_t[:, qt * 128 : (qt + 1) * 128],
                        zp[bb][t][:, :257],
                        start=(t == 0),
                        stop=(t == TT - 1),
                    )

            def chunk(bb, ch, pre=None):
                """One 512-query chunk: S^T tiles -> exp -> AV (interleaved),
                then 1/rowsum scale + output DMA. pre[t] = callables woven in
                after S-tile t."""
                cq = ch * 512
                gbase = bb * N_SEQ + cq
                av_ps = [
                    psO.tile([128, 512], f32, tag="o", name=f"av{qt}") for qt in range(4)
                ]
                pts = []
                for t in range(TT):
                    sps = psS.tile([128, 512], f32, tag="s", name="sps")
                    for d_ in range(2):
                        nc.tensor.matmul(
                            sps[:],
                            xnT[bb][d_][:, t * 128 : (t + 1) * 128],
                            yT[bb][d_][:, cq : cq + 512],
                            start=(d_ == 0),
                            stop=(d_ == 1),
                        )
                    pt_t = pt_pool.tile([128, 512], bf16, tag=f"pt{t}", name=f"pt{t}")
                    nc.scalar.activation(pt_t[:], sps[:], func=AF.Exp)
                    pts.append(pt_t)
                    if t > 0:
                        av_step(bb, t - 1, pts[t - 1], av_ps)
                    if pre is not None and t < len(pre):
                        for fn in pre[t]:
                            fn()
                av_step(bb, TT - 1, pts[TT - 1], av_ps)
                for qt in range(4):
                    rcp = stage_pool.tile([128, 1], f32, tag="rcp", name="rcp")
                    nc.vector.reciprocal(rcp[:], av_ps[qt][:, 256:257])
                    ob = stage_pool.tile([128, 256], f32, tag="ob", name="ob")
                    nc.vector.tensor_scalar(
                        ob[:], av_ps[qt][:, :256], scalar1=rcp[:], scalar2=None,
                        op0=ALU.mult,
                    )
                    nc.gpsimd.dma_start(
                        o_d[gbase + qt * 128 : gbase + (qt + 1) * 128, :], ob[:]
                    )

            # ---- prologue: batch-0 LN + transposes, first Y^T/Z builds ----
            for i in range(4):
                ln_chain(0, i)
            for i in range(2):
                nc.gpsimd.dma_start(m_sb[i][:], m_d[i * 128 : (i + 1) * 128, :])
                nc.gpsimd.dma_start(pT_sb[i][:], pT_d[i * 128 : (i + 1) * 128, :])
            for j in range(TT):
                if j + 4 < TT:
                    ln_chain(0, j + 4)
                ln_transpose(0, j)
                dummy_mm()
            yt_build(0, 0, 0)
            yt_build(0, 0, 1)
            z_build(0, 0)
            z_build(0, 1)

            # ---- main loop; remaining prep woven into the chunk PE stream ----
            def mk(f, *a):
                return lambda: f(*a)

            pre00 = [[] for _ in range(TT)]
            for t in range(TT - 2):  # z tiles 2..15, one per slot
                pre00[t].append(mk(z_build, 0, t + 2))
            for c in range(1, NCH):
                pre00[4 * c - 1].append(mk(yt_build, 0, c, 0))
                pre00[4 * c].append(mk(yt_build, 0, c, 1))
            pre01 = [[mk(ln_tile, 1, t)] for t in range(TT)]
            pre02 = [[mk(z_build, 1, t)] for t in range(TT)]
            pre03 = [[] for _ in range(TT)]
            for c in range(NCH):
                pre03[4 * c].append(mk(yt_build, 1, c, 0))
                pre03[4 * c + 2].append(mk(yt_build, 1, c, 1))
            pre = {(0, 0): pre00, (0, 1): pre01, (0, 2): pre02, (0, 3): pre03}
            for bb in range(B):
                for ch in range(NCH):
                    chunk(bb, ch, pre.get((bb, ch)))

    nc.compile()
    return nc


def get_nc():
    if "nc" not in _CACHE:
        _CACHE["nc"] = _build()
    return _CACHE["nc"]


def make_in_maps(x, gamma, Wq, Wk, Wv, Wo):
    bf = ml_dtypes.bfloat16
    g = 1.0 + gamma.astype(np.float64)
    x_flat = np.ascontiguousarray(x.reshape(N_TOK, D).astype(np.float32))
    Wq64, Wk64, Wv64, Wo64 = (a.astype(np.float64) for a in (Wq, Wk, Wv, Wo))
    in_maps = []
    for h in range(HEADS):
        sl = slice(h * DH, (h + 1) * DH)
        M = SCALE * (Wq64[sl].T @ Wk64[sl]) * g[:, None] * g[None, :]
        PT = ((Wo64[:, sl] @ Wv64[sl]) * g[None, :]).T
        in_maps.append(
            {
                "x": x_flat,
                "m": np.ascontiguousarray(M.astype(bf)),
                "pT": np.ascontiguousarray(PT.astype(bf)),
            }
        )
    return in_maps


def gather(results):
    acc = np.zeros((N_TOK, D), np.float32)
    for h in range(HEADS):
        acc += results[h]["o_part"]
    return acc.reshape(B, N_SEQ, D)


def kernel(x, gamma, Wq, Wk, Wv, Wo):
    from concourse import bass_utils

    x, gamma, Wq, Wk, Wv, Wo = (
        np.asarray(a) for a in (x, gamma, Wq, Wk, Wv, Wo)
    )
    nc = get_nc()
    in_maps = make_in_maps(x, gamma, Wq, Wk, Wv, Wo)
    res = bass_utils.run_bass_kernel_spmd(
        nc, in_maps, core_ids=list(range(HEADS))
    )
    return gather(res.results).astype(np.float32)
